# revision 16
# baseline (speedup 1.0000x reference)
"""Trainium2 Bass kernel for nn_CAL_51015621542567 (fused single-NEFF).

Cross-attention (D queries over T keys, L features) + gated residual +
LayerNorm(D) + ReLU + BatchNorm2d(train) + 1x1 conv + LayerNorm(D) + gate.

Data-parallel over batch: B=32 sharded as 4 batches on each of 8 NeuronCores.
ONE NEFF launch: stage 1 computes attention + pre-LN + ReLU per batch and
accumulates per-core BatchNorm partial stats; an on-device AllReduce
(gpsimd collective over DRAM bounce buffers) sums the [2, 768] partials
across the 8 cores; stage 2 applies BN (folded into bf16 1x1-conv weights),
the conv, post-LN and the gate.

Intermediate h is stored bf16: batches 0..SPILL-1 round-trip via HBM,
the last two stay resident in SBUF so stage 2 can start right after the
collective.  Big matmuls run f32r (1 cycle/row); the conv runs bf16.
The softmax normalization (gate_av/sumexp) is folded into the attention
transpose as a regular matmul against a bf16 diagonal matrix.
"""

import sys

sys.path.insert(0, "/opt/trn_rl_repo")

from contextlib import ExitStack

import numpy as np
import concourse.bass as bass
from concourse import bacc
import concourse.mybir as mybir
import concourse.tile as tile
from concourse.masks import make_identity
from concourse.bass_utils import run_bass_kernel_spmd

AF = mybir.ActivationFunctionType
OP = mybir.AluOpType

P = 128
B, D, T, L = 32, 768, 768, 1024
NCORES = 8
BL = B // NCORES  # batches per core
DT, TT, LT, OT = D // P, T // P, L // P, D // P
EPS = 1e-5
NB = float(B * L)  # BatchNorm stat count
SPILL = 2  # batches whose h round-trips via HBM (rest stay in SBUF)

f32 = mybir.dt.float32
f32r = mybir.dt.float32r
bf16 = mybir.dt.bfloat16

N1 = 384  # mm1 free-dim chunk (2 x 384 inside one [P, 2, 512] psum tile)
N2 = 512  # mm2/mm3 free-dim chunk over L (2 x 512)

PARAM_KEYS = [
    "gate_av", "gate", "ln_before_w", "ln_before_b", "bn_gamma",
    "bn_beta", "mlp_w", "ln_post_w", "ln_post_b",
]


def _half_rows(nc, rowp, psp, psSum, psSq, eps_col):
    """[1, N2] psum sum/sumsq over D -> (mu, rs) f32r rows for this half.

    Temps live in the (currently idle) psBm/psBr psum banks."""
    mu = rowp.tile([1, N2], f32r, tag="mu_h")
    nc.vector.tensor_scalar_mul(mu[:], psSum[:], 1.0 / D)
    tmp = psp.tile([1, N2], f32, tag="psBm", name="mu2row")
    nc.vector.tensor_tensor(tmp[:], mu[:].bitcast(f32), mu[:].bitcast(f32),
                            OP.mult)
    v = rowp.tile([1, N2], f32, tag="vrs_h")
    nc.vector.tensor_scalar_mul(v[:], psSq[:], 1.0 / D)
    nc.vector.tensor_tensor(v[:], v[:], tmp[:], OP.subtract)
    lg = psp.tile([1, N2], f32, tag="psBr", name="lgrow")
    nc.scalar.activation(lg[:], v[:], AF.Ln, bias=eps_col[:1, :])
    rs = rowp.tile([1, N2], f32r, tag="vrs_h")
    nc.scalar.activation(rs[:], lg[:], AF.Exp, scale=-0.5)
    return mu, rs


def build_fused():
    nc = bacc.Bacc("TRN2", target_bir_lowering=False, debug=False,
                   num_devices=NCORES)
    x_d = nc.dram_tensor("x", [BL, D, L], f32, kind="ExternalInput").ap()
    y_d = nc.dram_tensor("y", [BL, T, L], f32, kind="ExternalInput").ap()
    gav_d = nc.dram_tensor("gate_av", [1], f32, kind="ExternalInput").ap()
    g_d = nc.dram_tensor("gate", [1], f32, kind="ExternalInput").ap()
    lnw_d = nc.dram_tensor("ln_before_w", [D], f32, kind="ExternalInput").ap()
    lnb_d = nc.dram_tensor("ln_before_b", [D], f32, kind="ExternalInput").ap()
    bng_d = nc.dram_tensor("bn_gamma", [D], f32, kind="ExternalInput").ap()
    bnb_d = nc.dram_tensor("bn_beta", [D], f32, kind="ExternalInput").ap()
    w_d = nc.dram_tensor("mlp_w", [D, D], f32, kind="ExternalInput").ap()
    lpw_d = nc.dram_tensor("ln_post_w", [D], f32, kind="ExternalInput").ap()
    lpb_d = nc.dram_tensor("ln_post_b", [D], f32, kind="ExternalInput").ap()
    out_d = nc.dram_tensor("out", [BL, D, L], f32, kind="ExternalOutput").ap()

    with tile.TileContext(nc) as tc:
        with ExitStack() as st:
            cp = st.enter_context(tc.tile_pool(name="consts", bufs=1))
            xp = st.enter_context(tc.tile_pool(name="xp", bufs=2))
            yp = st.enter_context(tc.tile_pool(name="yp", bufs=2))
            tp = st.enter_context(tc.tile_pool(name="tp", bufs=1))
            hp = st.enter_context(tc.tile_pool(name="hp", bufs=2))
            ep = st.enter_context(tc.tile_pool(name="ep", bufs=1))
            sqp = st.enter_context(tc.tile_pool(name="sqp", bufs=2))
            rowp = st.enter_context(tc.tile_pool(name="rowp", bufs=1))
            smallp = st.enter_context(tc.tile_pool(name="small", bufs=2))
            psp = st.enter_context(tc.tile_pool(name="psum", bufs=1,
                                                space="PSUM"))
            dramp = st.enter_context(tc.tile_pool(name="dram", bufs=1,
                                                  space="DRAM"))

            hdram = dramp.tile([SPILL, D, L], bf16)
            cc_in = dramp.tile([2, D], f32)
            cc_out = dramp.tile([2, D], f32)

            # ---------------- constants / params ----------------
            # (f32r matmul operands must be produced as f32r: DMA into an
            #  f32r tile or an engine write to an f32r tile — the BIR
            #  verifier rejects plain bitcasts of f32-written SBUF.)
            ident_f = sqp.tile([P, P], f32, tag="sq")  # transient setup tile
            make_identity(nc, ident_f[:])
            ident = cp.tile([P, P], f32r, name="ident")
            nc.vector.tensor_copy(ident[:], ident_f[:])
            identr = ident[:]
            identb = cp.tile([P, P], bf16, name="identb")
            nc.vector.tensor_copy(identb[:], ident_f[:])

            ones_col_f = sqp.tile([P, 1], f32, tag="sq")  # transient
            nc.gpsimd.memset(ones_col_f[:], 1.0)
            ones_col_r = cp.tile([P, 1], f32r, name="ones_col_r")
            nc.vector.tensor_copy(ones_col_r[:], ones_col_f[:])
            ones_col = ones_col_r[:]
            ones_col_b = cp.tile([P, 1], bf16, name="ones_col_b")
            nc.gpsimd.memset(ones_col_b[:], 1.0)
            ones_row_f = cp.tile([1, P], f32, name="ones_row_f")
            nc.gpsimd.memset(ones_row_f[:], 1.0)
            ones_row_r = cp.tile([1, P], f32r, name="ones_row_r")
            nc.vector.tensor_copy(ones_row_r[:], ones_row_f[:])
            ones_row = ones_row_r[:]
            eps_col = cp.tile([P, 1], f32, name="eps_col")
            nc.gpsimd.memset(eps_col[:], EPS)

            def load_param(ap_d, pname):
                t = cp.tile([P, DT], f32, name=pname)
                nc.sync.dma_start(t[:], ap_d.rearrange("(t p) -> p t", p=P))
                return t

            lnw = load_param(lnw_d, "lnw")
            lnb = load_param(lnb_d, "lnb")
            bng = load_param(bng_d, "bng")
            bnb = load_param(bnb_d, "bnb")
            lpw = load_param(lpw_d, "lpw")
            lpb = load_param(lpb_d, "lpb")

            # gate_av, gate -> per-partition scalars via K=1 bcast matmul
            g2 = cp.tile([1, 2], f32, name="g2")
            nc.gpsimd.memset(g2[:], 0.0)
            nc.sync.dma_start(g2[:, 0:1], gav_d[None, :])
            nc.sync.dma_start(g2[:, 1:2], g_d[None, :])
            psG = psp.tile([P, 2], f32, tag="psBr", name="psG")
            nc.tensor.matmul(psG[:], ones_row_f[:], g2[:],
                             start=True, stop=True)
            gav_sb = cp.tile([P, 1], f32, name="gav_sb")
            nc.scalar.copy(gav_sb[:], psG[:, 0:1])
            g_sb = cp.tile([P, 1], f32, name="g_sb")
            nc.scalar.copy(g_sb[:], psG[:, 1:2])
            w2g = cp.tile([P, DT], f32, name="w2g")
            nc.vector.tensor_scalar_mul(w2g[:], lpw[:], g_sb[:])
            b2g = cp.tile([P, DT], f32, name="b2g")
            nc.vector.tensor_scalar_mul(b2g[:], lpb[:], g_sb[:])

            bn_slots = cp.tile([P, DT, 2 * BL], f32, name="bn_slots")
            bn2_slots = cp.tile([P, DT, 2 * BL], f32, name="bn2_slots")

            # ================= stage 1: attention + LN + ReLU =================
            hout_tiles = {}
            for b in range(BL):
                with nc.named_scope(f"s1_b{b}"):
                    x_nat = xp.tile([P, DT, L], f32r, tag="xnat")
                    y_nat = yp.tile([P, TT, L], f32r, tag="ynat")
                    xv = x_d[b].rearrange("(dt p) l -> p dt l", p=P)\
                        .bitcast(f32r)
                    yv = y_d[b].rearrange("(tt p) l -> p tt l", p=P)\
                        .bitcast(f32r)
                    for dt in range(DT):
                        nc.sync.dma_start(x_nat[:, dt], xv[:, dt])
                    for tt in range(TT):
                        nc.sync.dma_start(y_nat[:, tt], yv[:, tt])

                    # phase A: xT, yT via PE transposes (copies on DVE/Pool)
                    xT = tp.tile([P, LT, D], f32r, tag="xT")
                    yT = tp.tile([P, LT, T], f32r, tag="yT")
                    for lt in range(LT):
                        lsl = slice(lt * P, (lt + 1) * P)
                        psA = psp.tile([P, D], f32r, tag="psMM", bufs=2,
                                       name="psA")
                        for dt in range(DT):
                            nc.tensor.transpose(
                                psA[:, dt * P:(dt + 1) * P],
                                x_nat[:, dt, lsl], identr)
                        nc.vector.tensor_copy(xT[:, lt], psA[:].bitcast(f32))
                        psB_ = psp.tile([P, T], f32r, tag="psMM", bufs=2,
                                        name="psB_")
                        for tt in range(TT):
                            nc.tensor.transpose(
                                psB_[:, tt * P:(tt + 1) * P],
                                y_nat[:, tt, lsl], identr)
                        nc.scalar.copy(yT[:, lt], psB_[:].bitcast(f32))

                    # phase B: mm1 + softmax + fused scale-transpose
                    ET = tp.tile([P, TT, D], f32r, tag="ET")
                    for dt in range(DT):
                        dsl = slice(dt * P, (dt + 1) * P)
                        psS = psp.tile([P, 2, N2], f32, tag="psMM", bufs=2,
                                       name="psS")
                        for lt in range(LT):
                            for ch in range(2):
                                nsl = slice(ch * N1, (ch + 1) * N1)
                                nc.tensor.matmul(
                                    psS[:, ch, :N1], xT[:, lt, dsl],
                                    yT[:, lt, nsl], start=(lt == 0),
                                    stop=(lt == LT - 1))
                        negmax = smallp.tile([P, 1], f32, tag="negmax")
                        nc.vector.tensor_reduce(
                            negmax[:], psS[:, :, :N1],
                            axis=mybir.AxisListType.XY, op=OP.max,
                            negate=True)
                        E = ep.tile([P, T], bf16, tag="E")
                        sumexp = smallp.tile([P, 1], f32, tag="sumexp")
                        nc.scalar.activation(
                            E[:].rearrange("p (c n) -> p c n", c=2),
                            psS[:, :, :N1], AF.Exp, bias=negmax[:],
                            accum_out=sumexp[:])
                        rg = smallp.tile([P, 1], f32, tag="rg")
                        nc.vector.reciprocal(rg[:], sumexp[:])
                        nc.vector.tensor_scalar_mul(rg[:], rg[:], gav_sb[:])
                        diagb = smallp.tile([P, P], bf16, tag="diag")
                        nc.vector.tensor_scalar_mul(diagb[:], identb[:],
                                                    rg[:])
                        # attT block = E_blk^T @ diag(rg): regular bf16 matmul
                        psE = psp.tile([P, T], f32, tag="psMM", bufs=2,
                                       name="psE")
                        for tt in range(TT):
                            csl = slice(tt * P, (tt + 1) * P)
                            nc.tensor.matmul(psE[:, csl], E[:, csl],
                                             diagb[:], start=True, stop=True)
                        nc.scalar.copy(
                            ET[:, :, dsl],
                            psE[:].rearrange("p (t c) -> p t c", c=P))

                    # phase C1: mm2 + residual add (into x_nat in place)
                    for dt in range(DT):
                        dsl = slice(dt * P, (dt + 1) * P)
                        psR = psp.tile([P, 2, N2], f32, tag="psMM", bufs=2,
                                       name="psR")
                        for tt in range(TT):
                            for ch in range(2):
                                csl = slice(ch * N2, (ch + 1) * N2)
                                nc.tensor.matmul(
                                    psR[:, ch], ET[:, tt, dsl],
                                    y_nat[:, tt, csl], start=(tt == 0),
                                    stop=(tt == TT - 1))
                        hv3 = x_nat[:, dt].rearrange("p (c n) -> p c n", c=2)
                        nc.vector.tensor_tensor(hv3, hv3.bitcast(f32),
                                                psR[:], OP.add)

                    # phase C2: per-half LN stats + apply + ReLU -> hout bf16
                    hout_t = hp.tile([P, DT, L], bf16, tag="hout")
                    hout_tiles[b] = hout_t
                    for ch in range(2):
                        csl = slice(ch * N2, (ch + 1) * N2)
                        psLsum = psp.tile([1, N2], f32, tag="psLs",
                                          name="psLsum")
                        psLsq = psp.tile([1, N2], f32, tag="psLq",
                                         name="psLsq")
                        for dt in range(DT):
                            sq = sqp.tile([P, N2], bf16, tag="sq")
                            nc.gpsimd.tensor_tensor(
                                sq[:], x_nat[:, dt, csl].bitcast(f32),
                                x_nat[:, dt, csl].bitcast(f32), OP.mult)
                            nc.tensor.matmul(
                                psLsum[:], ones_col, x_nat[:, dt, csl],
                                start=(dt == 0), stop=(dt == DT - 1))
                            nc.tensor.matmul(
                                psLsq[:], ones_col_b[:], sq[:],
                                start=(dt == 0), stop=(dt == DT - 1))
                        mu, rs = _half_rows(nc, rowp, psp, psLsum, psLsq, eps_col)
                        psBmu = psp.tile([P, N2], f32, tag="psBm",
                                         name="psBmu")
                        nc.tensor.matmul(psBmu[:], ones_row, mu[:],
                                         start=True, stop=True)
                        psBrs = psp.tile([P, N2], f32, tag="psBr",
                                         name="psBrs")
                        nc.tensor.matmul(psBrs[:], ones_row, rs[:],
                                         start=True, stop=True)
                        # GpSimd cannot read PSUM: stage rs bcast into SBUF
                        rs_b = rowp.tile([P, N2], f32, tag="vrs_h")
                        nc.scalar.copy(rs_b[:], psBrs[:])
                        for dt in range(DT):
                            hv = x_nat[:, dt, csl]
                            nc.vector.tensor_tensor(hv, hv.bitcast(f32),
                                                    psBmu[:], OP.subtract)
                            nc.gpsimd.tensor_tensor(hv, hv.bitcast(f32),
                                                    rs_b[:], OP.mult)
                            slot = slice(2 * b + ch, 2 * b + ch + 1)
                            nc.scalar.activation(
                                hout_t[:, dt, csl], hv.bitcast(f32),
                                AF.Relu, scale=lnw[:, dt:dt + 1],
                                bias=lnb[:, dt:dt + 1],
                                accum_out=bn_slots[:, dt, slot])
                            sq2 = sqp.tile([P, N2], bf16, tag="sq")
                            nc.scalar.activation(
                                sq2[:], hout_t[:, dt, csl], AF.Square,
                                accum_out=bn2_slots[:, dt, slot])
                    if b < SPILL:
                        hv_d = hdram[:][b].rearrange("(dt p) l -> p dt l",
                                                     p=P)
                        for dt in range(DT):
                            nc.sync.dma_start(hv_d[:, dt], hout_t[:, dt])

            # ============ BN stat pack + on-device AllReduce ============
            with nc.named_scope("ccred"):
                bn_sum = cp.tile([P, DT], f32, name="bn_sum")
                nc.vector.tensor_reduce(bn_sum[:], bn_slots[:],
                                        axis=mybir.AxisListType.X, op=OP.add)
                bn_sq = cp.tile([P, DT], f32, name="bn_sq")
                nc.vector.tensor_reduce(bn_sq[:], bn2_slots[:],
                                        axis=mybir.AxisListType.X, op=OP.add)
                bnpack = cp.tile([P, 2, DT], f32, name="bnpack")
                nc.vector.tensor_copy(bnpack[:, 0], bn_sum[:])
                nc.vector.tensor_copy(bnpack[:, 1], bn_sq[:])
                nc.gpsimd.dma_start(
                    cc_in[:].rearrange("s (t p) -> p s t", p=P), bnpack[:])
                nc.gpsimd.collective_compute(
                    "AllReduce", OP.add,
                    replica_groups=[list(range(NCORES))],
                    ins=[cc_in[:].opt()], outs=[cc_out[:].opt()])
                gstats = cp.tile([P, 2, DT], f32, name="gstats")
                nc.gpsimd.dma_start(
                    gstats[:], cc_out[:].rearrange("s (t p) -> p s t", p=P))

            # ============ W transpose (fills the collective wait) ============
            with nc.named_scope("wprep"):
                w_nat = yp.tile([P, OT, D], f32r, tag="ynat")
                wv = w_d.rearrange("(ot p) d -> p ot d", p=P).bitcast(f32r)
                for ot in range(OT):
                    nc.sync.dma_start(w_nat[:, ot], wv[:, ot])
                WT = cp.tile([P, DT, D], bf16, name="WT")
                for dt in range(DT):
                    psW = psp.tile([P, D], f32r, tag="psMM", bufs=2,
                                   name="psW")
                    for ot in range(OT):
                        nc.tensor.transpose(
                            psW[:, ot * P:(ot + 1) * P],
                            w_nat[:, ot, dt * P:(dt + 1) * P], identr)
                    nc.scalar.copy(WT[:, dt], psW[:].bitcast(f32))

            # ============ BN finalize: fold into WT + cvec ============
            with nc.named_scope("bnfin"):
                mu_bn = cp.tile([P, DT], f32, name="mu_bn")
                nc.vector.tensor_scalar_mul(mu_bn[:], gstats[:, 0], 1.0 / NB)
                ex2 = cp.tile([P, DT], f32, name="ex2")
                nc.vector.tensor_scalar_mul(ex2[:], gstats[:, 1], 1.0 / NB)
                mu2 = cp.tile([P, DT], f32, name="mu2")
                nc.vector.tensor_tensor(mu2[:], mu_bn[:], mu_bn[:], OP.mult)
                var_bn = cp.tile([P, DT], f32, name="var_bn")
                nc.vector.tensor_tensor(var_bn[:], ex2[:], mu2[:],
                                        OP.subtract)
                lv = cp.tile([P, DT], f32, name="lv")
                nc.scalar.activation(lv[:], var_bn[:], AF.Ln, bias=eps_col[:])
                rs_bn = cp.tile([P, DT], f32, name="rs_bn")
                nc.scalar.activation(rs_bn[:], lv[:], AF.Exp, scale=-0.5)
                s_bn = cp.tile([P, DT], f32, name="s_bn")
                nc.vector.tensor_tensor(s_bn[:], bng[:], rs_bn[:], OP.mult)
                smu = cp.tile([P, DT], f32, name="smu")
                nc.vector.tensor_tensor(smu[:], s_bn[:], mu_bn[:], OP.mult)
                q = cp.tile([P, DT], f32, name="q")
                nc.vector.tensor_tensor(q[:], bnb[:], smu[:], OP.subtract)
                qb = cp.tile([P, DT], bf16, name="qb")
                nc.vector.tensor_copy(qb[:], q[:])

                # cvec[o] = sum_d W[o, d] * q[d]  (bf16, N=1 matmuls)
                psC = psp.tile([P, OT], f32, tag="psBm", name="psC")
                for ot in range(OT):
                    for dt in range(DT):
                        nc.tensor.matmul(
                            psC[:, ot:ot + 1],
                            WT[:, dt, ot * P:(ot + 1) * P],
                            qb[:, dt:dt + 1],
                            start=(dt == 0), stop=(dt == DT - 1))
                cvec = cp.tile([P, OT], f32, name="cvec")
                nc.scalar.copy(cvec[:], psC[:])
                for dt in range(DT):
                    nc.vector.tensor_scalar_mul(WT[:, dt], WT[:, dt],
                                                s_bn[:, dt:dt + 1])

            # ======== stage 2: conv + post-LN + gate (resident first) ========
            order = list(range(SPILL, BL)) + list(range(SPILL))
            for b in order:
                with nc.named_scope(f"s2_b{b}"):
                    if b < SPILL:
                        hr_t = hp.tile([P, DT, L], bf16, tag="hout")
                        hv_d = hdram[:][b].rearrange("(dt p) l -> p dt l",
                                                     p=P)
                        for dt in range(DT):
                            nc.sync.dma_start(hr_t[:, dt], hv_d[:, dt])
                    else:
                        hr_t = hout_tiles[b]
                    z = tp.tile([P, OT, L], f32r, tag="yT")
                    for ot in range(OT):
                        osl = slice(ot * P, (ot + 1) * P)
                        psZ = psp.tile([P, 2, N2], f32, tag="psMM", bufs=2,
                                       name="psZ")
                        for dt in range(DT):
                            for ch in range(2):
                                csl = slice(ch * N2, (ch + 1) * N2)
                                nc.tensor.matmul(
                                    psZ[:, ch], WT[:, dt, osl],
                                    hr_t[:, dt, csl], start=(dt == 0),
                                    stop=(dt == DT - 1))
                        nc.scalar.activation(
                            z[:, ot].rearrange("p (c n) -> p c n", c=2),
                            psZ[:], AF.Identity, bias=cvec[:, ot:ot + 1])
                    ov = out_d[b].rearrange("(ot p) l -> p ot l", p=P)
                    for ch in range(2):
                        csl = slice(ch * N2, (ch + 1) * N2)
                        psPsum = psp.tile([1, N2], f32, tag="psLs",
                                          name="psPsum")
                        psPsq = psp.tile([1, N2], f32, tag="psLq",
                                         name="psPsq")
                        for ot in range(OT):
                            sqz = sqp.tile([P, N2], bf16, tag="sq")
                            nc.gpsimd.tensor_tensor(
                                sqz[:], z[:, ot, csl].bitcast(f32),
                                z[:, ot, csl].bitcast(f32), OP.mult)
                            nc.tensor.matmul(
                                psPsum[:], ones_col, z[:, ot, csl],
                                start=(ot == 0), stop=(ot == OT - 1))
                            nc.tensor.matmul(
                                psPsq[:], ones_col_b[:], sqz[:],
                                start=(ot == 0), stop=(ot == OT - 1))
                        mu, rs = _half_rows(nc, rowp, psp, psPsum, psPsq, eps_col)
                        psBmu = psp.tile([P, N2], f32, tag="psBm",
                                         name="psBmu2")
                        nc.tensor.matmul(psBmu[:], ones_row, mu[:],
                                         start=True, stop=True)
                        psBrs = psp.tile([P, N2], f32, tag="psBr",
                                         name="psBrs2")
                        nc.tensor.matmul(psBrs[:], ones_row, rs[:],
                                         start=True, stop=True)
                        rs_b = rowp.tile([P, N2], f32, tag="vrs_h")
                        nc.scalar.copy(rs_b[:], psBrs[:])
                        for ot in range(OT):
                            zv = z[:, ot, csl]
                            nc.vector.tensor_tensor(zv, zv.bitcast(f32),
                                                    psBmu[:], OP.subtract)
                            nc.gpsimd.tensor_tensor(zv, zv.bitcast(f32),
                                                    rs_b[:], OP.mult)
                            osb = xp.tile([P, N2], f32, tag="xnat")
                            nc.scalar.activation(
                                osb[:], zv.bitcast(f32), AF.Identity,
                                scale=w2g[:, ot:ot + 1],
                                bias=b2g[:, ot:ot + 1])
                            nc.sync.dma_start(ov[:, ot, csl], osb[:])

    nc.compile()
    return nc


_PROGRAM = None


def _get_program():
    global _PROGRAM
    if _PROGRAM is None:
        _PROGRAM = build_fused()
    return _PROGRAM


def run_fused(x, y, params, trace=False):
    """Run the fused program; returns (out [B, D, L], exec_ns, scopes)."""
    nc = _get_program()
    ins = []
    for c in range(NCORES):
        in_map = {
            "x": x[c * BL:(c + 1) * BL],
            "y": y[c * BL:(c + 1) * BL],
        }
        for k in PARAM_KEYS:
            in_map[k] = params[k]
        ins.append(in_map)
    r = run_bass_kernel_spmd(nc, ins, core_ids=list(range(NCORES)),
                             trace=trace)
    out = np.concatenate([r.results[c]["out"] for c in range(NCORES)],
                         axis=0)
    return out, r.exec_time_ns, (r.per_core_scope_times or {})


def kernel(**inputs) -> np.ndarray:
    import concourse.bass_utils as bu

    bu.upload_artifacts = lambda d: d  # no artifact store in container

    x = np.ascontiguousarray(np.asarray(inputs["x"])[..., 0], dtype=np.float32)
    y = np.ascontiguousarray(np.asarray(inputs["y"])[..., 0], dtype=np.float32)
    params = {
        k: np.ascontiguousarray(np.asarray(inputs[k]), dtype=np.float32)
        for k in PARAM_KEYS
    }
    out, _, _ = run_fused(x, y, params)
    return out[..., None]


# revision 19
# speedup vs baseline: 1.1101x; 1.1101x over previous
"""Trainium2 Bass kernel for nn_CAL_51015621542567 (fused single-NEFF).

Cross-attention (D queries over T keys, L features) + gated residual +
LayerNorm(D) + ReLU + BatchNorm2d(train) + 1x1 conv + LayerNorm(D) + gate.

Data-parallel over batch: B=32 sharded as 4 batches on each of 8 NeuronCores.
ONE NEFF launch: stage 1 computes attention + pre-LN + ReLU per batch and
accumulates per-core BatchNorm partial stats; two on-device AllReduces
(batches 0..2 overlapped under batch 3's compute, then batch 3's tail)
sum the [2, 768] partials across the 8 cores; stage 2 applies BN (folded
into bf16 1x1-conv weights), the conv, post-LN and the gate.

Throughput notes: big matmuls run f32r (1 cycle/row), the conv runs bf16.
The softmax normalization (gate_av/sumexp) is folded into the attention
transpose as a regular bf16 matmul against a diagonal matrix.  The
attention-transpose matmuls are software-pipelined one step behind mm1 so
the tensor engine never waits on the softmax chain.  rsqrt is computed as
Sqrt + DVE reciprocal to avoid activation-table thrash (Ln/Exp reloads).
LN row temps live in psum; LN mu/rs broadcasts are staged into SBUF so
the apply passes are SBUF-only and can use the Pool (gpsimd) engine.
Intermediate h is stored bf16: batches 0..SPILL-1 round-trip via HBM, the
last two stay SBUF-resident so stage 2 starts right after the collective.
Stage-2 z double-buffers through the idle xT/yT slots so consecutive
batches' convs overlap the post-LN apply.
"""

import sys

sys.path.insert(0, "/opt/trn_rl_repo")

from contextlib import ExitStack

import numpy as np
import concourse.bass as bass
from concourse import bacc
import concourse.mybir as mybir
import concourse.tile as tile
from concourse.masks import make_identity
from concourse.bass_utils import run_bass_kernel_spmd

AF = mybir.ActivationFunctionType
OP = mybir.AluOpType

P = 128
B, D, T, L = 32, 768, 768, 1024
NCORES = 8
BL = B // NCORES  # batches per core
DT, TT, LT, OT = D // P, T // P, L // P, D // P
EPS = 1e-5
NB = float(B * L)  # BatchNorm stat count
SPILL = 2  # batches whose h round-trips via HBM (rest stay in SBUF)

f32 = mybir.dt.float32
f32r = mybir.dt.float32r
bf16 = mybir.dt.bfloat16

N1 = 384  # mm1 free-dim chunk (2 x 384 inside one [P, 2, 512] psum tile)
N2 = 512  # mm2/mm3 free-dim chunk over L (2 x 512)
CS = 480  # phase-A psum->SBUF copy split point (DVE | Scalar)

PARAM_KEYS = [
    "gate_av", "gate", "ln_before_w", "ln_before_b", "bn_gamma",
    "bn_beta", "mlp_w", "ln_post_w", "ln_post_b",
]


def build_fused():
    nc = bacc.Bacc("TRN2", target_bir_lowering=False, debug=False,
                   num_devices=NCORES)
    x_d = nc.dram_tensor("x", [BL, D, L], f32, kind="ExternalInput").ap()
    y_d = nc.dram_tensor("y", [BL, T, L], f32, kind="ExternalInput").ap()
    gav_d = nc.dram_tensor("gate_av", [1], f32, kind="ExternalInput").ap()
    g_d = nc.dram_tensor("gate", [1], f32, kind="ExternalInput").ap()
    lnw_d = nc.dram_tensor("ln_before_w", [D], f32, kind="ExternalInput").ap()
    lnb_d = nc.dram_tensor("ln_before_b", [D], f32, kind="ExternalInput").ap()
    bng_d = nc.dram_tensor("bn_gamma", [D], f32, kind="ExternalInput").ap()
    bnb_d = nc.dram_tensor("bn_beta", [D], f32, kind="ExternalInput").ap()
    w_d = nc.dram_tensor("mlp_w", [D, D], f32, kind="ExternalInput").ap()
    lpw_d = nc.dram_tensor("ln_post_w", [D], f32, kind="ExternalInput").ap()
    lpb_d = nc.dram_tensor("ln_post_b", [D], f32, kind="ExternalInput").ap()
    out_d = nc.dram_tensor("out", [BL, D, L], f32, kind="ExternalOutput").ap()

    with tile.TileContext(nc) as tc:
        with ExitStack() as st:
            cp = st.enter_context(tc.tile_pool(name="consts", bufs=1))
            xp = st.enter_context(tc.tile_pool(name="xp", bufs=2))
            yp = st.enter_context(tc.tile_pool(name="yp", bufs=2))
            tp = st.enter_context(tc.tile_pool(name="tp", bufs=1))
            hp = st.enter_context(tc.tile_pool(name="hp", bufs=2))
            ep = st.enter_context(tc.tile_pool(name="ep", bufs=1))
            sqp = st.enter_context(tc.tile_pool(name="sqp", bufs=2))
            rowp = st.enter_context(tc.tile_pool(name="rowp", bufs=1))
            smallp = st.enter_context(tc.tile_pool(name="small", bufs=2))
            psp = st.enter_context(tc.tile_pool(name="psum", bufs=1,
                                                space="PSUM"))
            dramp = st.enter_context(tc.tile_pool(name="dram", bufs=1,
                                                  space="DRAM"))

            hdram = dramp.tile([SPILL, D, L], bf16)
            ccA_in = dramp.tile([2, D], f32)
            ccA_out = dramp.tile([2, D], f32)
            ccB_in = dramp.tile([2, D], f32)
            ccB_out = dramp.tile([2, D], f32)

            # PSUM tags (8 banks):
            #   psMM  bufs=2, 4KB slot -> 4 banks (psA/psB_/psS/psR/psZ/psW)
            #   psX   bufs=1, 3KB slot -> 2 banks (psE, bcast tiles, psG)
            #   psStat bufs=2, 2KB slot -> 2 banks (LN sums + row temps, psC)

            # ---------------- constants / params ----------------
            # (f32r matmul operands must be produced as f32r: DMA into an
            #  f32r tile or an engine write to an f32r tile.)
            ident_f = sqp.tile([P, P], f32, tag="sq")  # transient setup tile
            make_identity(nc, ident_f[:])
            identr_t = cp.tile([P, P], f32r, name="ident")
            nc.vector.tensor_copy(identr_t[:], ident_f[:])
            identr = identr_t[:]
            identb = cp.tile([P, P], bf16, name="identb")
            nc.vector.tensor_copy(identb[:], ident_f[:])

            ones_col_f = sqp.tile([P, 1], f32, tag="sq")  # transient
            nc.gpsimd.memset(ones_col_f[:], 1.0)
            ones_col_r = cp.tile([P, 1], f32r, name="ones_col_r")
            nc.vector.tensor_copy(ones_col_r[:], ones_col_f[:])
            ones_col = ones_col_r[:]
            ones_col_b = cp.tile([P, 1], bf16, name="ones_col_b")
            nc.gpsimd.memset(ones_col_b[:], 1.0)
            ones_row_f = cp.tile([1, P], f32, name="ones_row_f")
            nc.gpsimd.memset(ones_row_f[:], 1.0)
            ones_row_r = cp.tile([1, P], f32r, name="ones_row_r")
            nc.vector.tensor_copy(ones_row_r[:], ones_row_f[:])
            ones_row = ones_row_r[:]
            eps_col = cp.tile([P, 1], f32, name="eps_col")
            nc.gpsimd.memset(eps_col[:], EPS)

            def load_param(ap_d, pname):
                t = cp.tile([P, DT], f32, name=pname)
                nc.sync.dma_start(t[:], ap_d.rearrange("(t p) -> p t", p=P))
                return t

            lnw = load_param(lnw_d, "lnw")
            lnb = load_param(lnb_d, "lnb")
            bng = load_param(bng_d, "bng")
            bnb = load_param(bnb_d, "bnb")
            lpw = load_param(lpw_d, "lpw")
            lpb = load_param(lpb_d, "lpb")

            # gate_av, gate -> per-partition scalars via K=1 bcast matmul
            g2 = cp.tile([1, 2], f32, name="g2")
            nc.gpsimd.memset(g2[:], 0.0)
            nc.sync.dma_start(g2[:, 0:1], gav_d[None, :])
            nc.sync.dma_start(g2[:, 1:2], g_d[None, :])
            psG = psp.tile([P, 2], f32, tag="psX", name="psG")
            nc.tensor.matmul(psG[:], ones_row_f[:], g2[:],
                             start=True, stop=True)
            gav_sb = cp.tile([P, 1], f32, name="gav_sb")
            nc.scalar.copy(gav_sb[:], psG[:, 0:1])
            g_sb = cp.tile([P, 1], f32, name="g_sb")
            nc.scalar.copy(g_sb[:], psG[:, 1:2])
            w2g = cp.tile([P, DT], f32, name="w2g")
            nc.vector.tensor_scalar_mul(w2g[:], lpw[:], g_sb[:])
            b2g = cp.tile([P, DT], f32, name="b2g")
            nc.vector.tensor_scalar_mul(b2g[:], lpb[:], g_sb[:])

            bn_slots = cp.tile([P, DT, 2 * BL], f32, name="bn_slots")
            bn2_slots = cp.tile([P, DT, 2 * BL], f32, name="bn2_slots")

            def rows_for_half(psSum, psSq):
                """psum [1,N2] sum/sumsq over D -> SBUF bcasts (mu_b, rs_b).

                rsqrt = Sqrt + DVE reciprocal (no Ln/Exp table reloads);
                temps rotate through the psStat banks."""
                mu = rowp.tile([1, N2], f32r, tag="mu_h")
                nc.vector.tensor_scalar_mul(mu[:], psSum[:], 1.0 / D)
                m2 = psp.tile([1, N2], f32, tag="psStat", name="m2row")
                nc.vector.tensor_tensor(m2[:], mu[:].bitcast(f32),
                                        mu[:].bitcast(f32), OP.mult)
                v = rowp.tile([1, N2], f32, tag="vrs_h")
                nc.vector.tensor_scalar_mul(v[:], psSq[:], 1.0 / D)
                nc.vector.tensor_tensor(v[:], v[:], m2[:], OP.subtract)
                sr = psp.tile([1, N2], f32, tag="psStat", name="srrow")
                nc.scalar.activation(sr[:], v[:], AF.Sqrt,
                                     bias=eps_col[:1, :])
                rs = rowp.tile([1, N2], f32r, tag="vrs_h")
                with nc.allow_low_precision(reason="f32r bits == f32"):
                    nc.vector.reciprocal(rs[:], sr[:])
                psBm = psp.tile([P, N2], f32, tag="psX", name="psBm")
                nc.tensor.matmul(psBm[:], ones_row, mu[:],
                                 start=True, stop=True)
                mu_b = rowp.tile([P, N2], f32, tag="mu_h")
                nc.vector.tensor_copy(mu_b[:], psBm[:])
                psBr = psp.tile([P, N2], f32, tag="psX", name="psBr")
                nc.tensor.matmul(psBr[:], ones_row, rs[:],
                                 start=True, stop=True)
                rs_b = rowp.tile([P, N2], f32, tag="vrs_h")
                nc.scalar.copy(rs_b[:], psBr[:])
                return mu_b, rs_b

            def collective(tag, cc_in, cc_out, slot_lo, slot_hi):
                """Reduce bn slots [slot_lo:slot_hi) and AllReduce them."""
                bsum = cp.tile([P, 2, DT], f32, name=f"bnp_{tag}")
                nc.vector.tensor_reduce(
                    bsum[:, 0], bn_slots[:, :, slot_lo:slot_hi],
                    axis=mybir.AxisListType.X, op=OP.add)
                nc.vector.tensor_reduce(
                    bsum[:, 1], bn2_slots[:, :, slot_lo:slot_hi],
                    axis=mybir.AxisListType.X, op=OP.add)
                nc.gpsimd.dma_start(
                    cc_in[:].rearrange("s (t p) -> p s t", p=P), bsum[:])
                nc.gpsimd.collective_compute(
                    "AllReduce", OP.add,
                    replica_groups=[list(range(NCORES))],
                    ins=[cc_in[:].opt()], outs=[cc_out[:].opt()])
                gst = cp.tile([P, 2, DT], f32, name=f"gst_{tag}")
                nc.gpsimd.dma_start(
                    gst[:], cc_out[:].rearrange("s (t p) -> p s t", p=P))
                return gst

            # ================= stage 1 =================
            hout_tiles = {}
            gstA = None
            for b in range(BL):
                with nc.named_scope(f"s1_b{b}"):
                    x_nat = xp.tile([P, DT, L], f32r, tag="xnat")
                    y_nat = yp.tile([P, TT, L], f32r, tag="ynat")
                    xv = x_d[b].rearrange("(dt p) l -> p dt l", p=P)\
                        .bitcast(f32r)
                    yv = y_d[b].rearrange("(tt p) l -> p tt l", p=P)\
                        .bitcast(f32r)
                    for dt in range(DT):
                        nc.sync.dma_start(x_nat[:, dt], xv[:, dt])
                    for tt in range(TT):
                        nc.sync.dma_start(y_nat[:, tt], yv[:, tt])

                    # phase A: xT, yT via PE transposes; copies split DVE|Sc
                    xT = tp.tile([P, LT, D], f32r, tag="xT")
                    yT = tp.tile([P, LT, T], f32r, tag="yT")
                    for lt in range(LT):
                        lsl = slice(lt * P, (lt + 1) * P)
                        psA = psp.tile([P, D], f32r, tag="psMM", bufs=2,
                                       name="psA")
                        for dt in range(DT):
                            nc.tensor.transpose(
                                psA[:, dt * P:(dt + 1) * P],
                                x_nat[:, dt, lsl], identr)
                        nc.vector.tensor_copy(xT[:, lt, 0:CS],
                                              psA[:, 0:CS].bitcast(f32))
                        nc.scalar.copy(xT[:, lt, CS:D],
                                       psA[:, CS:D].bitcast(f32))
                        psB_ = psp.tile([P, T], f32r, tag="psMM", bufs=2,
                                        name="psB_")
                        for tt in range(TT):
                            nc.tensor.transpose(
                                psB_[:, tt * P:(tt + 1) * P],
                                y_nat[:, tt, lsl], identr)
                        nc.vector.tensor_copy(yT[:, lt, 0:CS],
                                              psB_[:, 0:CS].bitcast(f32))
                        nc.scalar.copy(yT[:, lt, CS:T],
                                       psB_[:, CS:T].bitcast(f32))

                    # phase B: mm1 + softmax; attT matmuls skewed one dt
                    # behind mm1 so PE never waits on the softmax chain
                    ET = tp.tile([P, TT, D], f32r, tag="ET")

                    def flush_attT(pend):
                        E, diagb, dsl = pend
                        psE = psp.tile([P, T], f32, tag="psX", name="psE")
                        for tt in range(TT):
                            csl = slice(tt * P, (tt + 1) * P)
                            nc.tensor.matmul(psE[:, csl], E[:, csl],
                                             diagb[:], start=True, stop=True)
                        nc.scalar.copy(
                            ET[:, :, dsl],
                            psE[:].rearrange("p (t c) -> p t c", c=P))

                    pend = None
                    for dt in range(DT):
                        dsl = slice(dt * P, (dt + 1) * P)
                        psS = psp.tile([P, 2, N2], f32, tag="psMM", bufs=2,
                                       name="psS")
                        for lt in range(LT):
                            for ch in range(2):
                                nsl = slice(ch * N1, (ch + 1) * N1)
                                nc.tensor.matmul(
                                    psS[:, ch, :N1], xT[:, lt, dsl],
                                    yT[:, lt, nsl], start=(lt == 0),
                                    stop=(lt == LT - 1))
                        if pend is not None:
                            flush_attT(pend)
                        negmax = smallp.tile([P, 1], f32, tag="negmax")
                        nc.vector.tensor_reduce(
                            negmax[:], psS[:, :, :N1],
                            axis=mybir.AxisListType.XY, op=OP.max,
                            negate=True)
                        E = ep.tile([P, T], bf16, tag="E")
                        sumexp = smallp.tile([P, 1], f32, tag="sumexp")
                        nc.scalar.activation(
                            E[:].rearrange("p (c n) -> p c n", c=2),
                            psS[:, :, :N1], AF.Exp, bias=negmax[:],
                            accum_out=sumexp[:])
                        rg = smallp.tile([P, 1], f32, tag="rg")
                        nc.vector.reciprocal(rg[:], sumexp[:])
                        nc.vector.tensor_scalar_mul(rg[:], rg[:], gav_sb[:])
                        diagb = smallp.tile([P, P], bf16, tag="diag")
                        nc.vector.tensor_scalar_mul(diagb[:], identb[:],
                                                    rg[:])
                        pend = (E, diagb, dsl)
                    flush_attT(pend)

                    # phase C1: mm2 + residual add (into x_nat in place)
                    for dt in range(DT):
                        dsl = slice(dt * P, (dt + 1) * P)
                        psR = psp.tile([P, 2, N2], f32, tag="psMM", bufs=2,
                                       name="psR")
                        for tt in range(TT):
                            for ch in range(2):
                                csl = slice(ch * N2, (ch + 1) * N2)
                                nc.tensor.matmul(
                                    psR[:, ch], ET[:, tt, dsl],
                                    y_nat[:, tt, csl], start=(tt == 0),
                                    stop=(tt == TT - 1))
                        hv3 = x_nat[:, dt].rearrange("p (c n) -> p c n", c=2)
                        nc.vector.tensor_tensor(hv3, hv3.bitcast(f32),
                                                psR[:], OP.add)

                    # phase C2: per-half LN stats + apply + ReLU -> hout bf16
                    hout_t = hp.tile([P, DT, L], bf16, tag="hout")
                    hout_tiles[b] = hout_t
                    for ch in range(2):
                        csl = slice(ch * N2, (ch + 1) * N2)
                        psLsum = psp.tile([1, N2], f32, tag="psStat",
                                          name="psLsum")
                        psLsq = psp.tile([1, N2], f32, tag="psStat",
                                         name="psLsq")
                        for dt in range(DT):
                            sq = sqp.tile([P, N2], bf16, tag="sq")
                            nc.gpsimd.tensor_tensor(
                                sq[:], x_nat[:, dt, csl].bitcast(f32),
                                x_nat[:, dt, csl].bitcast(f32), OP.mult)
                            nc.tensor.matmul(
                                psLsum[:], ones_col, x_nat[:, dt, csl],
                                start=(dt == 0), stop=(dt == DT - 1))
                            nc.tensor.matmul(
                                psLsq[:], ones_col_b[:], sq[:],
                                start=(dt == 0), stop=(dt == DT - 1))
                        mu_b, rs_b = rows_for_half(psLsum, psLsq)
                        for dt in range(DT):
                            hv = x_nat[:, dt, csl]
                            nc.vector.tensor_tensor(hv, hv.bitcast(f32),
                                                    mu_b[:], OP.subtract)
                            nc.gpsimd.tensor_tensor(hv, hv.bitcast(f32),
                                                    rs_b[:], OP.mult)
                            slot = slice(2 * b + ch, 2 * b + ch + 1)
                            nc.scalar.activation(
                                hout_t[:, dt, csl], hv.bitcast(f32),
                                AF.Relu, scale=lnw[:, dt:dt + 1],
                                bias=lnb[:, dt:dt + 1],
                                accum_out=bn_slots[:, dt, slot])
                            sq2 = sqp.tile([P, N2], bf16, tag="sq")
                            nc.scalar.activation(
                                sq2[:], hout_t[:, dt, csl], AF.Square,
                                accum_out=bn2_slots[:, dt, slot])
                    if b < SPILL:
                        hv_d = hdram[:][b].rearrange("(dt p) l -> p dt l",
                                                     p=P)
                        for dt in range(DT):
                            nc.sync.dma_start(hv_d[:, dt], hout_t[:, dt])
                    if b == BL - 2:
                        # AllReduce for batches 0..BL-2 runs under batch
                        # BL-1's compute; only batch BL-1's tail is exposed
                        with nc.named_scope("ccA"):
                            gstA = collective("A", ccA_in, ccA_out,
                                              0, 2 * (BL - 1))

            with nc.named_scope("ccB"):
                gstB = collective("B", ccB_in, ccB_out,
                                  2 * (BL - 1), 2 * BL)

            # ============ W transpose (fills the collective wait) ============
            with nc.named_scope("wprep"):
                w_nat = yp.tile([P, OT, D], f32r, tag="ynat")
                wv = w_d.rearrange("(ot p) d -> p ot d", p=P).bitcast(f32r)
                for ot in range(OT):
                    nc.sync.dma_start(w_nat[:, ot], wv[:, ot])
                WT = cp.tile([P, DT, D], bf16, name="WT")
                for dt in range(DT):
                    psW = psp.tile([P, D], f32r, tag="psMM", bufs=2,
                                   name="psW")
                    for ot in range(OT):
                        nc.tensor.transpose(
                            psW[:, ot * P:(ot + 1) * P],
                            w_nat[:, ot, dt * P:(dt + 1) * P], identr)
                    nc.scalar.copy(WT[:, dt], psW[:].bitcast(f32))

            # ============ BN finalize: fold into WT + cvec ============
            with nc.named_scope("bnfin"):
                gstats = cp.tile([P, 2, DT], f32, name="gstats")
                nc.vector.tensor_tensor(gstats[:], gstA[:], gstB[:], OP.add)
                mu_bn = cp.tile([P, DT], f32, name="mu_bn")
                nc.vector.tensor_scalar_mul(mu_bn[:], gstats[:, 0], 1.0 / NB)
                ex2 = cp.tile([P, DT], f32, name="ex2")
                nc.vector.tensor_scalar_mul(ex2[:], gstats[:, 1], 1.0 / NB)
                mu2 = cp.tile([P, DT], f32, name="mu2")
                nc.vector.tensor_tensor(mu2[:], mu_bn[:], mu_bn[:], OP.mult)
                var_bn = cp.tile([P, DT], f32, name="var_bn")
                nc.vector.tensor_tensor(var_bn[:], ex2[:], mu2[:],
                                        OP.subtract)
                sq_bn = cp.tile([P, DT], f32, name="sq_bn")
                nc.scalar.activation(sq_bn[:], var_bn[:], AF.Sqrt,
                                     bias=eps_col[:])
                rs_bn = cp.tile([P, DT], f32, name="rs_bn")
                nc.vector.reciprocal(rs_bn[:], sq_bn[:])
                s_bn = cp.tile([P, DT], f32, name="s_bn")
                nc.vector.tensor_tensor(s_bn[:], bng[:], rs_bn[:], OP.mult)
                smu = cp.tile([P, DT], f32, name="smu")
                nc.vector.tensor_tensor(smu[:], s_bn[:], mu_bn[:], OP.mult)
                q = cp.tile([P, DT], f32, name="q")
                nc.vector.tensor_tensor(q[:], bnb[:], smu[:], OP.subtract)
                qb = cp.tile([P, DT], bf16, name="qb")
                nc.vector.tensor_copy(qb[:], q[:])

                # cvec[o] = sum_d W[o, d] * q[d]  (bf16, N=1 matmuls)
                psC = psp.tile([P, OT], f32, tag="psStat", name="psC")
                for ot in range(OT):
                    for dt in range(DT):
                        nc.tensor.matmul(
                            psC[:, ot:ot + 1],
                            WT[:, dt, ot * P:(ot + 1) * P],
                            qb[:, dt:dt + 1],
                            start=(dt == 0), stop=(dt == DT - 1))
                cvec = cp.tile([P, OT], f32, name="cvec")
                nc.scalar.copy(cvec[:], psC[:])
                for dt in range(DT):
                    nc.vector.tensor_scalar_mul(WT[:, dt], WT[:, dt],
                                                s_bn[:, dt:dt + 1])

            # ======== stage 2: conv + post-LN + gate (resident first) ========
            order = list(range(SPILL, BL)) + list(range(SPILL))
            for bi, b in enumerate(order):
                with nc.named_scope(f"s2_b{b}"):
                    if b < SPILL:
                        hr_t = hp.tile([P, DT, L], bf16, tag="hout")
                        hv_d = hdram[:][b].rearrange("(dt p) l -> p dt l",
                                                     p=P)
                        for dt in range(DT):
                            nc.sync.dma_start(hr_t[:, dt], hv_d[:, dt])
                    else:
                        hr_t = hout_tiles[b]
                    # z double-buffers across batches via idle xT/yT slots
                    z = tp.tile([P, OT, L], f32r,
                                tag=("yT" if bi % 2 == 0 else "xT"))
                    for ot in range(OT):
                        osl = slice(ot * P, (ot + 1) * P)
                        psZ = psp.tile([P, 2, N2], f32, tag="psMM", bufs=2,
                                       name="psZ")
                        for dt in range(DT):
                            for ch in range(2):
                                csl = slice(ch * N2, (ch + 1) * N2)
                                nc.tensor.matmul(
                                    psZ[:, ch], WT[:, dt, osl],
                                    hr_t[:, dt, csl], start=(dt == 0),
                                    stop=(dt == DT - 1))
                        nc.scalar.activation(
                            z[:, ot].rearrange("p (c n) -> p c n", c=2),
                            psZ[:], AF.Identity, bias=cvec[:, ot:ot + 1])
                    ov = out_d[b].rearrange("(ot p) l -> p ot l", p=P)
                    for ch in range(2):
                        csl = slice(ch * N2, (ch + 1) * N2)
                        psPsum = psp.tile([1, N2], f32, tag="psStat",
                                          name="psPsum")
                        psPsq = psp.tile([1, N2], f32, tag="psStat",
                                         name="psPsq")
                        for ot in range(OT):
                            sqz = sqp.tile([P, N2], bf16, tag="sq")
                            nc.gpsimd.tensor_tensor(
                                sqz[:], z[:, ot, csl].bitcast(f32),
                                z[:, ot, csl].bitcast(f32), OP.mult)
                            nc.tensor.matmul(
                                psPsum[:], ones_col, z[:, ot, csl],
                                start=(ot == 0), stop=(ot == OT - 1))
                            nc.tensor.matmul(
                                psPsq[:], ones_col_b[:], sqz[:],
                                start=(ot == 0), stop=(ot == OT - 1))
                        mu_b, rs_b = rows_for_half(psPsum, psPsq)
                        for ot in range(OT):
                            zv = z[:, ot, csl]
                            nc.vector.tensor_tensor(zv, zv.bitcast(f32),
                                                    mu_b[:], OP.subtract)
                            nc.gpsimd.tensor_tensor(zv, zv.bitcast(f32),
                                                    rs_b[:], OP.mult)
                            osb = xp.tile([P, N2], f32, tag="xnat")
                            nc.scalar.activation(
                                osb[:], zv.bitcast(f32), AF.Identity,
                                scale=w2g[:, ot:ot + 1],
                                bias=b2g[:, ot:ot + 1])
                            nc.sync.dma_start(ov[:, ot, csl], osb[:])

    nc.compile()
    return nc


_PROGRAM = None


def _get_program():
    global _PROGRAM
    if _PROGRAM is None:
        _PROGRAM = build_fused()
    return _PROGRAM


def run_fused(x, y, params, trace=False):
    """Run the fused program; returns (out [B, D, L], exec_ns, scopes)."""
    nc = _get_program()
    ins = []
    for c in range(NCORES):
        in_map = {
            "x": x[c * BL:(c + 1) * BL],
            "y": y[c * BL:(c + 1) * BL],
        }
        for k in PARAM_KEYS:
            in_map[k] = params[k]
        ins.append(in_map)
    r = run_bass_kernel_spmd(nc, ins, core_ids=list(range(NCORES)),
                             trace=trace)
    out = np.concatenate([r.results[c]["out"] for c in range(NCORES)],
                         axis=0)
    return out, r.exec_time_ns, (r.per_core_scope_times or {})


def kernel(**inputs) -> np.ndarray:
    import concourse.bass_utils as bu

    bu.upload_artifacts = lambda d: d  # no artifact store in container

    x = np.ascontiguousarray(np.asarray(inputs["x"])[..., 0], dtype=np.float32)
    y = np.ascontiguousarray(np.asarray(inputs["y"])[..., 0], dtype=np.float32)
    params = {
        k: np.ascontiguousarray(np.asarray(inputs[k]), dtype=np.float32)
        for k in PARAM_KEYS
    }
    out, _, _ = run_fused(x, y, params)
    return out[..., None]


# revision 32
# speedup vs baseline: 1.2271x; 1.1053x over previous
"""Trainium2 Bass kernel for nn_CAL_51015621542567 (fused single-NEFF).

Cross-attention (D queries over T keys, L features) + gated residual +
LayerNorm(D) + ReLU + BatchNorm2d(train) + 1x1 conv + LayerNorm(D) + gate.

Data-parallel over batch: B=32 sharded as 4 batches on each of 8 NeuronCores.
ONE NEFF launch; two on-device AllReduces for the BatchNorm stats (batches
0..2 overlapped under batch 3's compute, then batch 3's small tail).

The kernel is macro-pipelined: slot b interleaves batch b's LayerNorm
stats/apply (vector/scalar/pool-heavy) with batch b+1's attention matmuls
and batch b+2's input transposes (tensor-engine-heavy), so the PE stays
busy and ramps to full p-state.  Within phase B the attention-transpose
matmuls are skewed one step behind mm1.  Other key tricks:
 - softmax normalization (gate_av/sumexp) folded into the attention
   transpose as a regular bf16 matmul against a diagonal matrix;
 - rsqrt = Sqrt + reciprocal_approx_fast (no Ln/Exp act-table thrash,
   no 3.3us serial DVE reciprocal);
 - LN apply runs in bf16 (h-mu -> hout) which frees x_nat early for the
   next-next batch's DMA and doubles elementwise throughput;
 - LN mu/rs broadcasts staged into SBUF so the apply can use the Pool
   engine (which cannot touch PSUM);
 - conv weights folded with the BN scale in bf16; h stored bf16 with the
   last two batches SBUF-resident (no HBM round trip).
"""

import sys

sys.path.insert(0, "/opt/trn_rl_repo")

from contextlib import ExitStack

import numpy as np
import concourse.bass as bass
from concourse import bacc
import concourse.mybir as mybir
import concourse.tile as tile
from concourse.masks import make_identity
from concourse.bass_utils import run_bass_kernel_spmd

AF = mybir.ActivationFunctionType
OP = mybir.AluOpType

P = 128
B, D, T, L = 32, 768, 768, 1024
NCORES = 8
BL = B // NCORES  # batches per core
DT, TT, LT, OT = D // P, T // P, L // P, D // P
EPS = 1e-5
NB = float(B * L)  # BatchNorm stat count
SPILL = 2  # batches whose h round-trips via HBM (rest stay in SBUF)

f32 = mybir.dt.float32
f32r = mybir.dt.float32r
bf16 = mybir.dt.bfloat16

N1 = 384  # mm1 free-dim chunk (2 x 384 inside one [P, 2, 512] psum tile)
N2 = 512  # mm2/mm3 free-dim chunk over L (2 x 512)
CS = 480  # phase-A psum->SBUF copy split point (DVE | Scalar)

PARAM_KEYS = [
    "gate_av", "gate", "ln_before_w", "ln_before_b", "bn_gamma",
    "bn_beta", "mlp_w", "ln_post_w", "ln_post_b",
]


def build_fused():
    nc = bacc.Bacc("TRN2", target_bir_lowering=False, debug=False,
                   num_devices=NCORES)
    x_d = nc.dram_tensor("x", [BL, D, L], f32, kind="ExternalInput").ap()
    y_d = nc.dram_tensor("y", [BL, T, L], f32, kind="ExternalInput").ap()
    gav_d = nc.dram_tensor("gate_av", [1], f32, kind="ExternalInput").ap()
    g_d = nc.dram_tensor("gate", [1], f32, kind="ExternalInput").ap()
    lnw_d = nc.dram_tensor("ln_before_w", [D], f32, kind="ExternalInput").ap()
    lnb_d = nc.dram_tensor("ln_before_b", [D], f32, kind="ExternalInput").ap()
    bng_d = nc.dram_tensor("bn_gamma", [D], f32, kind="ExternalInput").ap()
    bnb_d = nc.dram_tensor("bn_beta", [D], f32, kind="ExternalInput").ap()
    w_d = nc.dram_tensor("mlp_w", [D, D], f32, kind="ExternalInput").ap()
    lpw_d = nc.dram_tensor("ln_post_w", [D], f32, kind="ExternalInput").ap()
    lpb_d = nc.dram_tensor("ln_post_b", [D], f32, kind="ExternalInput").ap()
    out_d = nc.dram_tensor("out", [BL, D, L], f32, kind="ExternalOutput").ap()

    with tile.TileContext(nc) as tc:
        with ExitStack() as st:
            cp = st.enter_context(tc.tile_pool(name="consts", bufs=1))
            xp = st.enter_context(tc.tile_pool(name="xp", bufs=2))
            yp = st.enter_context(tc.tile_pool(name="yp", bufs=2))
            tp = st.enter_context(tc.tile_pool(name="tp", bufs=1))
            hp = st.enter_context(tc.tile_pool(name="hp", bufs=2))
            ep = st.enter_context(tc.tile_pool(name="ep", bufs=1))
            sqp = st.enter_context(tc.tile_pool(name="sqp", bufs=1))
            rowp = st.enter_context(tc.tile_pool(name="rowp", bufs=1))
            smallp = st.enter_context(tc.tile_pool(name="small", bufs=2))
            psp = st.enter_context(tc.tile_pool(name="psum", bufs=1,
                                                space="PSUM"))
            dramp = st.enter_context(tc.tile_pool(name="dram", bufs=1,
                                                  space="DRAM"))

            hdram = dramp.tile([SPILL, D, L], bf16)
            ccA_in = dramp.tile([2, D], f32)
            ccA_out = dramp.tile([2, D], f32)
            ccB_in = dramp.tile([2, D], f32)
            ccB_out = dramp.tile([2, D], f32)

            # PSUM tags (8 banks):
            #   psMM  bufs=2, 4KB slot -> 4 banks (psA/psB_/psS/psR/psZ/psW)
            #   psX   bufs=1, 3KB slot -> 2 banks (psE, bcast tiles, psG)
            #   psStat bufs=2, 2KB slot -> 2 banks (LN sums + row temps, psC)

            # ---------------- constants / params ----------------
            ident_f = sqp.tile([P, P], f32, tag="sq", bufs=2)  # transient setup tile
            make_identity(nc, ident_f[:])
            identr_t = cp.tile([P, P], f32r, name="ident")
            nc.vector.tensor_copy(identr_t[:], ident_f[:])
            identr = identr_t[:]
            identb = cp.tile([P, P], bf16, name="identb")
            nc.vector.tensor_copy(identb[:], ident_f[:])

            ones_col_f = sqp.tile([P, 1], f32, tag="sq", bufs=2)  # transient
            nc.gpsimd.memset(ones_col_f[:], 1.0)
            ones_col_r = cp.tile([P, 1], f32r, name="ones_col_r")
            nc.vector.tensor_copy(ones_col_r[:], ones_col_f[:])
            ones_col = ones_col_r[:]
            ones_col_b = cp.tile([P, 1], bf16, name="ones_col_b")
            nc.gpsimd.memset(ones_col_b[:], 1.0)
            ones_row_f = cp.tile([1, P], f32, name="ones_row_f")
            nc.gpsimd.memset(ones_row_f[:], 1.0)
            ones_row_r = cp.tile([1, P], f32r, name="ones_row_r")
            nc.vector.tensor_copy(ones_row_r[:], ones_row_f[:])
            ones_row = ones_row_r[:]
            eps_col = cp.tile([P, 1], f32, name="eps_col")
            nc.gpsimd.memset(eps_col[:], EPS)

            def load_param(ap_d, pname):
                t = cp.tile([P, DT], f32, name=pname)
                nc.sync.dma_start(t[:], ap_d.rearrange("(t p) -> p t", p=P))
                return t

            lnw = load_param(lnw_d, "lnw")
            lnb = load_param(lnb_d, "lnb")
            bng = load_param(bng_d, "bng")
            bnb = load_param(bnb_d, "bnb")
            lpw = load_param(lpw_d, "lpw")
            lpb = load_param(lpb_d, "lpb")

            g2 = cp.tile([1, 2], f32, name="g2")
            nc.gpsimd.memset(g2[:], 0.0)
            nc.sync.dma_start(g2[:, 0:1], gav_d[None, :])
            nc.sync.dma_start(g2[:, 1:2], g_d[None, :])
            psG = psp.tile([P, 2], f32, tag="psX", name="psG")
            nc.tensor.matmul(psG[:], ones_row_f[:], g2[:],
                             start=True, stop=True)
            gav_sb = cp.tile([P, 1], f32, name="gav_sb")
            nc.scalar.copy(gav_sb[:], psG[:, 0:1])
            g_sb = cp.tile([P, 1], f32, name="g_sb")
            nc.scalar.copy(g_sb[:], psG[:, 1:2])
            w2g = cp.tile([P, DT], f32, name="w2g")
            nc.vector.tensor_scalar_mul(w2g[:], lpw[:], g_sb[:])
            b2g = cp.tile([P, DT], f32, name="b2g")
            nc.vector.tensor_scalar_mul(b2g[:], lpb[:], g_sb[:])

            bn_slots = cp.tile([P, DT, 2 * BL], f32, name="bn_slots")
            bn2_slots = cp.tile([P, DT, 2 * BL], f32, name="bn2_slots")

            # ------------- reusable phase emitters -------------
            xn = {}   # b -> x_nat tile (holds x, then h)
            yn = {}   # b -> y_nat tile
            hout_tiles = {}

            def phase_A(b):
                """DMA x/y for batch b and build xT/yT transposes."""
                x_nat = xp.tile([P, DT, L], f32r, tag="xnat")
                y_nat = yp.tile([P, TT, L], f32r, tag="ynat")
                xn[b], yn[b] = x_nat, y_nat
                xv = x_d[b].rearrange("(dt p) l -> p dt l", p=P).bitcast(f32r)
                yv = y_d[b].rearrange("(tt p) l -> p tt l", p=P).bitcast(f32r)
                for dt in range(DT):
                    nc.sync.dma_start(x_nat[:, dt], xv[:, dt])
                for tt in range(TT):
                    nc.sync.dma_start(y_nat[:, tt], yv[:, tt])
                xT = tp.tile([P, LT, D], f32r, tag="xT")
                yT = tp.tile([P, LT, T], f32r, tag="yT")
                for lt in range(LT):
                    lsl = slice(lt * P, (lt + 1) * P)
                    psA = psp.tile([P, D], f32r, tag="psMM", bufs=2,
                                   name="psA")
                    for dt in range(DT):
                        nc.tensor.transpose(psA[:, dt * P:(dt + 1) * P],
                                            x_nat[:, dt, lsl], identr)
                    nc.vector.tensor_copy(xT[:, lt, 0:CS],
                                          psA[:, 0:CS].bitcast(f32))
                    nc.scalar.copy(xT[:, lt, CS:D], psA[:, CS:D].bitcast(f32))
                    psB_ = psp.tile([P, T], f32r, tag="psMM", bufs=2,
                                    name="psB_")
                    for tt in range(TT):
                        nc.tensor.transpose(psB_[:, tt * P:(tt + 1) * P],
                                            y_nat[:, tt, lsl], identr)
                    nc.vector.tensor_copy(yT[:, lt, 0:CS],
                                          psB_[:, 0:CS].bitcast(f32))
                    nc.scalar.copy(yT[:, lt, CS:T], psB_[:, CS:T].bitcast(f32))
                return xT, yT

            class PhaseB:
                """mm1 + softmax for batch b; attT matmuls skewed one dt."""

                def __init__(self, b, xT, yT):
                    self.xT, self.yT = xT, yT
                    self.ET = tp.tile([P, TT, D], f32r, tag="ET")
                    self.pend = None

                def _flush(self):
                    E, diagb, dsl = self.pend
                    self.pend = None
                    psE = psp.tile([P, T], f32, tag="psX", name="psE")
                    for tt in range(TT):
                        csl = slice(tt * P, (tt + 1) * P)
                        nc.tensor.matmul(psE[:, csl], E[:, csl], diagb[:],
                                         start=True, stop=True)
                    nc.scalar.copy(
                        self.ET[:, :, dsl],
                        psE[:].rearrange("p (t c) -> p t c", c=P))

                def emit(self, lo, hi):
                    for dt in range(lo, hi):
                        dsl = slice(dt * P, (dt + 1) * P)
                        psS = psp.tile([P, 2, N2], f32, tag="psMM", bufs=2,
                                       name="psS")
                        for lt in range(LT):
                            for ch in range(2):
                                nsl = slice(ch * N1, (ch + 1) * N1)
                                nc.tensor.matmul(
                                    psS[:, ch, :N1], self.xT[:, lt, dsl],
                                    self.yT[:, lt, nsl], start=(lt == 0),
                                    stop=(lt == LT - 1))
                        if self.pend is not None:
                            self._flush()
                        negmax = smallp.tile([P, 1], f32, tag="negmax")
                        nc.vector.tensor_reduce(
                            negmax[:], psS[:, :, :N1],
                            axis=mybir.AxisListType.XY, op=OP.max,
                            negate=True)
                        E = ep.tile([P, T], bf16, tag="E")
                        sumexp = smallp.tile([P, 1], f32, tag="sumexp")
                        nc.scalar.activation(
                            E[:].rearrange("p (c n) -> p c n", c=2),
                            psS[:, :, :N1], AF.Exp, bias=negmax[:],
                            accum_out=sumexp[:])
                        rg = smallp.tile([P, 1], f32, tag="rg")
                        nc.vector.reciprocal(rg[:], sumexp[:])
                        nc.vector.tensor_scalar_mul(rg[:], rg[:], gav_sb[:])
                        diagb = smallp.tile([P, P], bf16, tag="diag")
                        nc.vector.tensor_scalar_mul(diagb[:], identb[:],
                                                    rg[:])
                        self.pend = (E, diagb, slice(dt * P, (dt + 1) * P))

                def finish(self):
                    if self.pend is not None:
                        self._flush()

            def phase_C1(b, ET):
                """mm2 + residual add -> h in x_nat(b), in place."""
                x_nat, y_nat = xn[b], yn[b]
                for dt in range(DT):
                    dsl = slice(dt * P, (dt + 1) * P)
                    psR = psp.tile([P, 2, N2], f32, tag="psMM", bufs=2,
                                   name="psR")
                    for tt in range(TT):
                        for ch in range(2):
                            csl = slice(ch * N2, (ch + 1) * N2)
                            nc.tensor.matmul(
                                psR[:, ch], ET[:, tt, dsl],
                                y_nat[:, tt, csl], start=(tt == 0),
                                stop=(tt == TT - 1))
                    hv3 = x_nat[:, dt].rearrange("p (c n) -> p c n", c=2)
                    nc.vector.tensor_tensor(hv3, hv3.bitcast(f32), psR[:],
                                            OP.add)

            def emit_sqs(b, ch):
                """h^2 tiles for the LN variance, split DVE/Pool by parity."""
                x_nat = xn[b]
                csl = slice(ch * N2, (ch + 1) * N2)
                sqs = []
                for dt in range(DT):
                    sq = sqp.tile([P, N2], bf16, tag="sq", bufs=2)
                    eng = nc.gpsimd if dt % 2 == 0 else nc.vector
                    eng.tensor_tensor(sq[:], x_nat[:, dt, csl].bitcast(f32),
                                      x_nat[:, dt, csl].bitcast(f32),
                                      OP.mult)
                    sqs.append(sq)
                return sqs

            def emit_stats(b, ch, sqs):
                x_nat = xn[b]
                csl = slice(ch * N2, (ch + 1) * N2)
                psLsum = psp.tile([1, N2], f32, tag="psStat", bufs=2, name="psLsum")
                psLsq = psp.tile([1, N2], f32, tag="psStat", bufs=2, name="psLsq")
                for dt in range(DT):
                    nc.tensor.matmul(psLsum[:], ones_col, x_nat[:, dt, csl],
                                     start=(dt == 0), stop=(dt == DT - 1))
                    nc.tensor.matmul(psLsq[:], ones_col_b[:], sqs[dt][:],
                                     start=(dt == 0), stop=(dt == DT - 1))
                return psLsum, psLsq

            def rows_bcast(psSum, psSq, rs_dt=bf16):
                """-> (psBm [P,N2] f32 PSUM bcast, rs_b [P,N2] bf16 SBUF).

                The mu broadcast stays in psum (DVE subtract reads it
                directly); the rs broadcast is staged to SBUF bf16 so the
                Pool engine (no psum access) can do the multiply."""
                mu = rowp.tile([1, N2], f32r, tag="mu_h")
                nc.vector.tensor_scalar_mul(mu[:], psSum[:], 1.0 / D)
                m2 = psp.tile([1, N2], f32, tag="psStat", bufs=2, name="m2row")
                nc.vector.tensor_tensor(m2[:], mu[:].bitcast(f32),
                                        mu[:].bitcast(f32), OP.mult)
                v = rowp.tile([1, N2], f32, tag="vrs_h")
                nc.vector.tensor_scalar_mul(v[:], psSq[:], 1.0 / D)
                nc.vector.tensor_tensor(v[:], v[:], m2[:], OP.subtract)
                sr = psp.tile([1, N2], f32, tag="psStat", bufs=2, name="srrow")
                nc.scalar.activation(sr[:], v[:], AF.Sqrt, bias=eps_col[:1, :])
                rs_f = rowp.tile([1, N2], f32, tag="vrs_h")
                nc.vector.reciprocal_approx_fast(rs_f[:], sr[:])
                psBm = psp.tile([P, N2], f32, tag="psX", name="psBm")
                nc.tensor.matmul(psBm[:], ones_row, mu[:],
                                 start=True, stop=True)
                psBr = psp.tile([P, N2], f32, tag="psStat", bufs=2, name="psBr")
                nc.tensor.matmul(psBr[:], ones_row_f[:], rs_f[:],
                                 start=True, stop=True)
                rs_b = rowp.tile([P, N2], rs_dt, tag="vrs_h")
                nc.scalar.copy(rs_b[:], psBr[:])
                return psBm, rs_b

            def emit_apply(b, ch, psBm, rs_b, hout_t):
                """hout = relu(lnw*(h-mu)*rs + lnb) in bf16 + BN accums."""
                x_nat = xn[b]
                csl = slice(ch * N2, (ch + 1) * N2)
                for dt in range(DT):
                    hv = x_nat[:, dt, csl]
                    ho = hout_t[:, dt, csl]
                    nc.vector.tensor_tensor(ho, hv.bitcast(f32), psBm[:],
                                            OP.subtract)
                    nc.gpsimd.tensor_tensor(ho, ho, rs_b[:], OP.mult)
                    slot = slice(2 * b + ch, 2 * b + ch + 1)
                    nc.scalar.activation(ho, ho, AF.Relu,
                                         scale=lnw[:, dt:dt + 1],
                                         bias=lnb[:, dt:dt + 1],
                                         accum_out=bn_slots[:, dt, slot])
                    sq2 = sqp.tile([P, N2], bf16, tag="sqd", bufs=1)
                    nc.scalar.activation(sq2[:], ho, AF.Square,
                                         accum_out=bn2_slots[:, dt, slot])

            def collective(tag, cc_in, cc_out, slot_lo, slot_hi):
                bsum = cp.tile([P, 2, DT], f32, name=f"bnp_{tag}")
                nc.vector.tensor_reduce(
                    bsum[:, 0], bn_slots[:, :, slot_lo:slot_hi],
                    axis=mybir.AxisListType.X, op=OP.add)
                nc.vector.tensor_reduce(
                    bsum[:, 1], bn2_slots[:, :, slot_lo:slot_hi],
                    axis=mybir.AxisListType.X, op=OP.add)
                nc.gpsimd.dma_start(
                    cc_in[:].rearrange("s (t p) -> p s t", p=P), bsum[:])
                nc.gpsimd.collective_compute(
                    "AllReduce", OP.add,
                    replica_groups=[list(range(NCORES))],
                    ins=[cc_in[:].opt()], outs=[cc_out[:].opt()])
                gst = cp.tile([P, 2, DT], f32, name=f"gst_{tag}")
                nc.gpsimd.dma_start(
                    gst[:], cc_out[:].rearrange("s (t p) -> p s t", p=P))
                return gst

            # ================= stage 1: macro-pipelined slots =================
            with nc.named_scope("prologue"):
                xT0, yT0 = phase_A(0)
                pb = PhaseB(0, xT0, yT0)
                pb.emit(0, DT)
                pb.finish()
                phase_C1(0, pb.ET)
                nextTs = phase_A(1)

            gstA = None
            for b in range(BL):
                with nc.named_scope(f"slot{b}"):
                    nb = PhaseB(b + 1, *nextTs) if b + 1 < BL else None
                    hout_t = hp.tile([P, DT, L], bf16, tag="hout")
                    hout_tiles[b] = hout_t
                    sqs0 = emit_sqs(b, 0)
                    ps0 = emit_stats(b, 0, sqs0)
                    psBm0, rs_b0 = rows_bcast(*ps0)
                    emit_apply(b, 0, psBm0, rs_b0, hout_t)
                    if nb:
                        nb.emit(0, 2)
                    sqs1 = emit_sqs(b, 1)
                    ps1 = emit_stats(b, 1, sqs1)
                    psBm1, rs_b1 = rows_bcast(*ps1)
                    emit_apply(b, 1, psBm1, rs_b1, hout_t)
                    if nb:
                        nb.emit(2, DT)
                        nb.finish()
                    if b < SPILL:
                        hv_d = hdram[:][b].rearrange("(dt p) l -> p dt l",
                                                     p=P)
                        for dt in range(DT):
                            nc.sync.dma_start(hv_d[:, dt], hout_t[:, dt])
                    if b == BL - 2:
                        with nc.named_scope("ccA"):
                            gstA = collective("A", ccA_in, ccA_out,
                                              0, 2 * (BL - 1))
                    if nb:
                        phase_C1(b + 1, nb.ET)
                    if b + 2 < BL:
                        nextTs = phase_A(b + 2)

            with nc.named_scope("ccB"):
                gstB = collective("B", ccB_in, ccB_out,
                                  2 * (BL - 1), 2 * BL)

            # ============ W transpose (fills the collective wait) ============
            with nc.named_scope("wprep"):
                w_nat = yp.tile([P, OT, D], f32r, tag="ynat")
                wv = w_d.rearrange("(ot p) d -> p ot d", p=P).bitcast(f32r)
                for ot in range(OT):
                    nc.sync.dma_start(w_nat[:, ot], wv[:, ot])
                WT = cp.tile([P, DT, D], bf16, name="WT")
                for dt in range(DT):
                    psW = psp.tile([P, D], f32r, tag="psMM", bufs=2,
                                   name="psW")
                    for ot in range(OT):
                        nc.tensor.transpose(
                            psW[:, ot * P:(ot + 1) * P],
                            w_nat[:, ot, dt * P:(dt + 1) * P], identr)
                    nc.scalar.copy(WT[:, dt], psW[:].bitcast(f32))

            # ============ BN finalize: fold into WT + cvec ============
            with nc.named_scope("bnfin"):
                gstats = cp.tile([P, 2, DT], f32, name="gstats")
                nc.vector.tensor_tensor(gstats[:], gstA[:], gstB[:], OP.add)
                mu_bn = cp.tile([P, DT], f32, name="mu_bn")
                nc.vector.tensor_scalar_mul(mu_bn[:], gstats[:, 0], 1.0 / NB)
                ex2 = cp.tile([P, DT], f32, name="ex2")
                nc.vector.tensor_scalar_mul(ex2[:], gstats[:, 1], 1.0 / NB)
                mu2 = cp.tile([P, DT], f32, name="mu2")
                nc.vector.tensor_tensor(mu2[:], mu_bn[:], mu_bn[:], OP.mult)
                var_bn = cp.tile([P, DT], f32, name="var_bn")
                nc.vector.tensor_tensor(var_bn[:], ex2[:], mu2[:],
                                        OP.subtract)
                sq_bn = cp.tile([P, DT], f32, name="sq_bn")
                nc.scalar.activation(sq_bn[:], var_bn[:], AF.Sqrt,
                                     bias=eps_col[:])
                rs_bn = cp.tile([P, DT], f32, name="rs_bn")
                nc.vector.reciprocal(rs_bn[:], sq_bn[:])
                s_bn = cp.tile([P, DT], f32, name="s_bn")
                nc.vector.tensor_tensor(s_bn[:], bng[:], rs_bn[:], OP.mult)
                smu = cp.tile([P, DT], f32, name="smu")
                nc.vector.tensor_tensor(smu[:], s_bn[:], mu_bn[:], OP.mult)
                q = cp.tile([P, DT], f32, name="q")
                nc.vector.tensor_tensor(q[:], bnb[:], smu[:], OP.subtract)
                qb = cp.tile([P, DT], bf16, name="qb")
                nc.vector.tensor_copy(qb[:], q[:])

                psC = psp.tile([P, OT], f32, tag="psStat", bufs=2, name="psC")
                for ot in range(OT):
                    for dt in range(DT):
                        nc.tensor.matmul(
                            psC[:, ot:ot + 1],
                            WT[:, dt, ot * P:(ot + 1) * P],
                            qb[:, dt:dt + 1],
                            start=(dt == 0), stop=(dt == DT - 1))
                cvec = cp.tile([P, OT], f32, name="cvec")
                nc.scalar.copy(cvec[:], psC[:])
                for dt in range(DT):
                    nc.vector.tensor_scalar_mul(WT[:, dt], WT[:, dt],
                                                s_bn[:, dt:dt + 1])

            # ======== stage 2: conv + post-LN, skewed one batch ========
            order = list(range(SPILL, BL)) + list(range(SPILL))
            zs = {}

            def conv(bi, b):
                if b < SPILL:
                    hr_t = hp.tile([P, DT, L], bf16, tag="hout")
                    hv_d = hdram[:][b].rearrange("(dt p) l -> p dt l", p=P)
                    for dt in range(DT):
                        nc.sync.dma_start(hr_t[:, dt], hv_d[:, dt])
                else:
                    hr_t = hout_tiles[b]
                z = tp.tile([P, OT, L], f32r,
                            tag=("yT" if bi % 2 == 0 else "xT"))
                zs[b] = z
                for ot in range(OT):
                    osl = slice(ot * P, (ot + 1) * P)
                    psZ = psp.tile([P, 2, N2], f32, tag="psMM", bufs=2,
                                   name="psZ")
                    for dt in range(DT):
                        for ch in range(2):
                            csl = slice(ch * N2, (ch + 1) * N2)
                            nc.tensor.matmul(
                                psZ[:, ch], WT[:, dt, osl],
                                hr_t[:, dt, csl], start=(dt == 0),
                                stop=(dt == DT - 1))
                    nc.scalar.activation(
                        z[:, ot].rearrange("p (c n) -> p c n", c=2),
                        psZ[:], AF.Identity, bias=cvec[:, ot:ot + 1])

            def postln(b):
                z = zs[b]
                ov = out_d[b].rearrange("(ot p) l -> p ot l", p=P)
                for ch in range(2):
                    csl = slice(ch * N2, (ch + 1) * N2)
                    psPsum = psp.tile([1, N2], f32, tag="psStat",
                                      bufs=2, name="psPsum")
                    psPsq = psp.tile([1, N2], f32, tag="psStat",
                                     bufs=2, name="psPsq")
                    for ot in range(OT):
                        sqz = sqp.tile([P, N2], bf16, tag="sq", bufs=2)
                        eng = nc.gpsimd if ot % 2 == 0 else nc.vector
                        eng.tensor_tensor(sqz[:], z[:, ot, csl].bitcast(f32),
                                          z[:, ot, csl].bitcast(f32),
                                          OP.mult)
                        nc.tensor.matmul(psPsum[:], ones_col, z[:, ot, csl],
                                         start=(ot == 0), stop=(ot == OT - 1))
                        nc.tensor.matmul(psPsq[:], ones_col_b[:], sqz[:],
                                         start=(ot == 0), stop=(ot == OT - 1))
                    psBm, rs_b = rows_bcast(psPsum, psPsq, rs_dt=f32)
                    for ot in range(OT):
                        zv = z[:, ot, csl]
                        nc.vector.tensor_tensor(zv, zv.bitcast(f32),
                                                psBm[:], OP.subtract)
                        nc.gpsimd.tensor_tensor(zv, zv.bitcast(f32),
                                                rs_b[:], OP.mult)
                        osb = xp.tile([P, N2], f32, tag="xnat")
                        nc.scalar.activation(osb[:], zv.bitcast(f32),
                                             AF.Identity,
                                             scale=w2g[:, ot:ot + 1],
                                             bias=b2g[:, ot:ot + 1])
                        nc.sync.dma_start(ov[:, ot, csl], osb[:])

            for bi, b in enumerate(order):
                with nc.named_scope(f"s2_b{b}"):
                    conv(bi, b)
                    if bi > 0:
                        postln(order[bi - 1])
            with nc.named_scope("s2_tail"):
                postln(order[-1])

    nc.compile()
    return nc


_PROGRAM = None


def _get_program():
    global _PROGRAM
    if _PROGRAM is None:
        _PROGRAM = build_fused()
    return _PROGRAM


def run_fused(x, y, params, trace=False):
    """Run the fused program; returns (out [B, D, L], exec_ns, scopes)."""
    nc = _get_program()
    ins = []
    for c in range(NCORES):
        in_map = {
            "x": x[c * BL:(c + 1) * BL],
            "y": y[c * BL:(c + 1) * BL],
        }
        for k in PARAM_KEYS:
            in_map[k] = params[k]
        ins.append(in_map)
    r = run_bass_kernel_spmd(nc, ins, core_ids=list(range(NCORES)),
                             trace=trace)
    out = np.concatenate([r.results[c]["out"] for c in range(NCORES)],
                         axis=0)
    return out, r.exec_time_ns, (r.per_core_scope_times or {})


def kernel(**inputs) -> np.ndarray:
    import concourse.bass_utils as bu

    bu.upload_artifacts = lambda d: d  # no artifact store in container

    x = np.ascontiguousarray(np.asarray(inputs["x"])[..., 0], dtype=np.float32)
    y = np.ascontiguousarray(np.asarray(inputs["y"])[..., 0], dtype=np.float32)
    params = {
        k: np.ascontiguousarray(np.asarray(inputs[k]), dtype=np.float32)
        for k in PARAM_KEYS
    }
    out, _, _ = run_fused(x, y, params)
    return out[..., None]
